# revision 8
# baseline (speedup 1.0000x reference)
"""Additive attention kernel for 8 TRN2 NeuronCores (Bass/Tile).

Problem (S=1024, B=64, D=1024 for all of dec/align/enc):
    sa_s   = s_tm1 @ sa_w.T + sa_b                  # [B, A]
    h      = tanh(sa_s[None] + uh)                  # [S, B, A]
    e      = einsum('sba,a->sb', h, a1_w[0]) + a1_b # [S, B]
    e_ij   = masked softmax of e over S             # [S, B]
    attend = einsum('sb,sbd->bd', e_ij, xs_h)       # [B, D]
    returns (e_ij, attend)

Sharding: data-parallel over B — each of the 8 cores handles 8 batches.
Device layout choice: the align dim A lives on SBUF partitions so that
  - the bias add (sa_s) fuses into the ACT tanh as a per-partition bias,
  - the A-reduction (dot with a1_w) is a PE matvec (lhsT = w chunk [128,1]),
  - the S-reduction (attend) is a PE matvec (lhsT = e_ij column [128,1]).
Host-side sharding pre-transposes uh to (b, a, s) and xs_h to (b, s, d) so all
device DMAs are dense 4 KiB-per-partition line loads.
"""

import sys

if "/opt/trn_rl_repo" not in sys.path:
    sys.path.insert(0, "/opt/trn_rl_repo")

import numpy as np

S, B, D = 1024, 64, 1024
NCORES = 8
BL = B // NCORES  # 8 batches per core
NCH = D // 128    # 8 partition chunks of the align dim

_compiled_nc = None


def _build_nc():
    from contextlib import ExitStack

    import concourse.bacc as bacc
    import concourse.tile as tile
    from concourse import mybir

    f32 = mybir.dt.float32
    AF = mybir.ActivationFunctionType

    nc = bacc.Bacc(
        "TRN2",
        target_bir_lowering=False,
        debug=False,
        enable_asserts=False,
        num_devices=NCORES,
    )

    # Inputs (per core, already sharded/transposed on host).
    uht = nc.dram_tensor("uht", [BL, D, S], f32, kind="ExternalInput").ap()  # (b,a,s)
    xsh = nc.dram_tensor("xsh", [BL, S, D], f32, kind="ExternalInput").ap()  # (b,s,d)
    satw = nc.dram_tensor("satw", [D, D], f32, kind="ExternalInput").ap()  # sa_w.T (d,a)
    stm1t = nc.dram_tensor("stm1t", [D, BL], f32, kind="ExternalInput").ap()  # (d,b)
    sabt = nc.dram_tensor("sabt", [128, NCH], f32, kind="ExternalInput").ap()
    a1wt = nc.dram_tensor("a1wt", [128, NCH], f32, kind="ExternalInput").ap()
    maskt = nc.dram_tensor("maskt", [BL, S], f32, kind="ExternalInput").ap()  # (b,s)
    ident = nc.dram_tensor("ident", [128, 128], f32, kind="ExternalInput").ap()
    # Outputs.
    eijt = nc.dram_tensor("eijt", [BL, S], f32, kind="ExternalOutput").ap()  # (b,s)
    att = nc.dram_tensor("att", [BL, D], f32, kind="ExternalOutput").ap()  # (b,d)

    with tile.TileContext(nc) as tc, ExitStack() as ctx:
        singles = ctx.enter_context(tc.tile_pool(name="singles", bufs=1))
        ubig = ctx.enter_context(tc.tile_pool(name="ubig", bufs=2))
        xbig = ctx.enter_context(tc.tile_pool(name="xbig", bufs=2))
        small = ctx.enter_context(tc.tile_pool(name="small", bufs=1))
        psum = ctx.enter_context(tc.tile_pool(name="psum", bufs=2, space="PSUM"))

        # ---- constants ----------------------------------------------------
        satw_sb = singles.tile([128, NCH, D], f32, tag="satw")  # [d_p, d_k, a]
        nc.sync.dma_start(out=satw_sb, in_=satw.rearrange("(k p) a -> p k a", p=128))
        stm1t_sb = singles.tile([128, NCH, BL], f32, tag="stm1t")  # [d_p, d_k, b]
        nc.sync.dma_start(out=stm1t_sb, in_=stm1t.rearrange("(k p) b -> p k b", p=128))
        sabt_sb = singles.tile([128, NCH], f32, tag="sabt")
        nc.sync.dma_start(out=sabt_sb, in_=sabt)
        a1wt_sb = singles.tile([128, NCH], f32, tag="a1wt")
        nc.sync.dma_start(out=a1wt_sb, in_=a1wt)
        ident_sb = singles.tile([128, 128], f32, tag="ident")
        nc.sync.dma_start(out=ident_sb, in_=ident)
        maskt_sb = singles.tile([BL, S], f32, tag="maskt")
        nc.sync.dma_start(out=maskt_sb, in_=maskt)

        # ---- sa_s^T = (s_tm1 @ sa_w.T).T + sa_b, laid out [a_p, ac*BL+b] --
        psum_sas = psum.tile([128, NCH * BL], f32, tag="ps", name="psum_sas")
        for ac in range(NCH):
            for k in range(NCH):
                nc.tensor.matmul(
                    psum_sas[:, ac * BL : (ac + 1) * BL],
                    lhsT=satw_sb[:, k, ac * 128 : (ac + 1) * 128],
                    rhs=stm1t_sb[:, k, :],
                    start=(k == 0),
                    stop=(k == NCH - 1),
                )
        sas_sb = singles.tile([128, NCH * BL], f32, tag="sas")
        for ac in range(NCH):
            nc.scalar.activation(
                sas_sb[:, ac * BL : (ac + 1) * BL],
                psum_sas[:, ac * BL : (ac + 1) * BL],
                AF.Identity,
                bias=sabt_sb[:, ac : ac + 1],
                scale=1.0,
            )

        # ---- e rows: tanh + PE matvec over A ------------------------------
        # Row for batch b lands in psum_e[b // 4] at partition 32*(b % 4).
        psum_e = [
            psum.tile([128, S], f32, tag="pe", name=f"psum_e{i}") for i in range(2)
        ]
        for b in range(BL):
            ut = ubig.tile([128, NCH, S], f32, tag="U", name="ut")  # [a_p, ac, s]
            nc.sync.dma_start(out=ut, in_=uht[b].rearrange("(c p) s -> p c s", p=128))
            for c in range(NCH):
                nc.scalar.activation(
                    ut[:, c, :],
                    ut[:, c, :],
                    AF.Tanh,
                    bias=sas_sb[:, c * BL + b : c * BL + b + 1],
                    scale=1.0,
                )
            pt = psum_e[b // 4]
            row = 32 * (b % 4)
            for c in range(NCH):
                for h in range(2):
                    nc.tensor.matmul(
                        pt[row : row + 1, h * 512 : (h + 1) * 512],
                        lhsT=a1wt_sb[:, c : c + 1],
                        rhs=ut[:, c, h * 512 : (h + 1) * 512],
                        start=(c == 0),
                        stop=(c == NCH - 1),
                        tile_position=(0, row),
                    )

        # Gather the 8 e rows (at partitions {0,32,64,96} of two PSUM tiles)
        # into e_all[0:8]: full-width copy to SBUF, then partition-strided DMA.
        e_all = small.tile([BL, S], f32, tag="eall")
        for i in range(2):
            scr = small.tile([128, S], f32, tag="scr", bufs=2, name="scr")
            for j in range(4):
                nc.scalar.copy(
                    scr[32 * j : 32 * j + 1, :], psum_e[i][32 * j : 32 * j + 1, :]
                )
            strided = scr.rearrange("(a b) s -> a b s", b=32)[:, 0, :]  # [4, S]
            nc.sync.dma_start(out=e_all[4 * i : 4 * i + 4, :], in_=strided)

        # ---- masked softmax over S (free dim) -----------------------------
        mx = small.tile([BL, 1], f32, tag="mx")
        nc.vector.reduce_max(out=mx, in_=e_all, axis=mybir.AxisListType.X)
        nmx = small.tile([BL, 1], f32, tag="nmx")
        nc.vector.tensor_scalar_mul(nmx, mx, -1.0)
        ex = small.tile([BL, S], f32, tag="ex")
        nc.scalar.activation(ex, e_all, AF.Exp, bias=nmx, scale=1.0)
        exm = small.tile([BL, S], f32, tag="exm")
        nc.vector.tensor_mul(exm, ex, maskt_sb)
        sm = small.tile([BL, 1], f32, tag="sm")
        nc.vector.reduce_sum(out=sm, in_=exm, axis=mybir.AxisListType.X)
        rs = small.tile([BL, 1], f32, tag="rs")
        nc.vector.reciprocal(rs, sm)
        eij = small.tile([BL, S], f32, tag="eij")
        nc.vector.tensor_scalar_mul(eij, exm, rs)
        nc.sync.dma_start(out=eijt, in_=eij)

        # ---- transpose e_ij -> [s_p, c*BL+b] columns for the attend matvec
        psum_t = psum.tile([128, NCH * BL], f32, tag="ps", name="psum_t")
        for c in range(NCH):
            nc.tensor.transpose(
                psum_t[:, c * BL : (c + 1) * BL],
                eij[:, c * 128 : (c + 1) * 128],
                ident_sb[0:BL, 0:BL],
            )
        ecols = small.tile([128, NCH * BL], f32, tag="ecols")
        nc.vector.tensor_copy(ecols, psum_t)

        # ---- attend: PE matvec over S -------------------------------------
        psum_a = [
            psum.tile([128, D], f32, tag="pe", name=f"psum_a{i}") for i in range(2)
        ]
        for b in range(BL):
            xt = xbig.tile([128, NCH, D], f32, tag="X", name="xt")  # [s_p, sc, d]
            nc.sync.dma_start(out=xt, in_=xsh[b].rearrange("(c p) d -> p c d", p=128))
            pt = psum_a[b // 4]
            row = 32 * (b % 4)
            for c in range(NCH):
                for h in range(2):
                    nc.tensor.matmul(
                        pt[row : row + 1, h * 512 : (h + 1) * 512],
                        lhsT=ecols[:, c * BL + b : c * BL + b + 1],
                        rhs=xt[:, c, h * 512 : (h + 1) * 512],
                        start=(c == 0),
                        stop=(c == NCH - 1),
                        tile_position=(0, row),
                    )
        for i in range(2):
            scra = small.tile([128, D], f32, tag="scr", bufs=2, name="scra")
            for j in range(4):
                nc.scalar.copy(
                    scra[32 * j : 32 * j + 1, :], psum_a[i][32 * j : 32 * j + 1, :]
                )
            strided = scra.rearrange("(a b) d -> a b d", b=32)[:, 0, :]  # [4, D]
            nc.sync.dma_start(out=att[4 * i : 4 * i + 4, :], in_=strided)

    nc.compile()
    return nc


def get_nc():
    global _compiled_nc
    if _compiled_nc is None:
        _compiled_nc = _build_nc()
    return _compiled_nc


def make_in_maps(s_tm1, xs_h, uh, xs_mask, sa_w, sa_b, a1_w, a1_b):
    s_tm1 = np.asarray(s_tm1, dtype=np.float32)
    xs_h = np.asarray(xs_h, dtype=np.float32)
    uh = np.asarray(uh, dtype=np.float32)
    xs_mask = np.asarray(xs_mask, dtype=np.float32)
    sa_w = np.asarray(sa_w, dtype=np.float32)
    sa_b = np.asarray(sa_b, dtype=np.float32)
    a1_w = np.asarray(a1_w, dtype=np.float32)

    satw = np.ascontiguousarray(sa_w.T)
    sabt = np.ascontiguousarray(sa_b.reshape(NCH, 128).T)
    a1wt = np.ascontiguousarray(a1_w[0].reshape(NCH, 128).T)
    ident = np.eye(128, dtype=np.float32)

    in_maps = []
    for i in range(NCORES):
        bs = slice(i * BL, (i + 1) * BL)
        in_maps.append(
            {
                "uht": np.ascontiguousarray(uh[:, bs, :].transpose(1, 2, 0)),
                "xsh": np.ascontiguousarray(xs_h[:, bs, :].transpose(1, 0, 2)),
                "satw": satw,
                "stm1t": np.ascontiguousarray(s_tm1[bs].T),
                "sabt": sabt,
                "a1wt": a1wt,
                "maskt": np.ascontiguousarray(xs_mask[:, bs].T),
                "ident": ident,
            }
        )
    return in_maps


def assemble_outputs(results):
    e_ij = np.empty((S, B), dtype=np.float32)
    attend = np.empty((B, D), dtype=np.float32)
    for i, r in enumerate(results):
        e_ij[:, i * BL : (i + 1) * BL] = r["eijt"].T
        attend[i * BL : (i + 1) * BL, :] = r["att"]
    return e_ij, attend


def kernel(**inputs):
    from concourse.bass_utils import run_bass_kernel_spmd

    nc = get_nc()
    in_maps = make_in_maps(**inputs)
    res = run_bass_kernel_spmd(nc, in_maps, list(range(NCORES)))
    return assemble_outputs(res.results)


# revision 22
# speedup vs baseline: 1.3190x; 1.3190x over previous
"""Additive attention kernel for 8 TRN2 NeuronCores (Bass/Tile).

Problem (S=1024, B=64, D=1024 for all of dec/align/enc):
    sa_s   = s_tm1 @ sa_w.T + sa_b                  # [B, A]
    h      = tanh(sa_s[None] + uh)                  # [S, B, A]
    e      = einsum('sba,a->sb', h, a1_w[0]) + a1_b # [S, B]
    e_ij   = masked softmax of e over S             # [S, B]
    attend = einsum('sb,sbd->bd', e_ij, xs_h)       # [B, D]
    returns (e_ij, attend)

Sharding: data-parallel over B — each of the 8 cores handles 8 batches.
Device layout: the align dim A lives on SBUF partitions so the bias add fuses
into the ACT tanh as a per-partition bias, and both reductions (dot with a1_w
over A; weighted sum over S) are PE matvecs in float32r (1 cycle/row vs 4 for
fp32).

float32r matmuls reject column tile_position offsets, so each batch's matvec
uses M=4 stationary weights with the weight vector in column b%4 and zeros
elsewhere: row b%4 of the [4, N] PSUM output gets the real dot product, the
other rows accumulate zeros. The 4 rows of each PSUM tile land at partitions
0..3, which every later engine op can address (SBUF engine operands must start
at partition 0/32/64/96).

Host-side sharding pre-transposes uh to (b, a, s) and xs_h to (b, s, d) so all
device DMAs are dense 4 KiB-per-partition line loads; small outputs are
returned in device-friendly layouts and rearranged on host.
"""

import sys

if "/opt/trn_rl_repo" not in sys.path:
    sys.path.insert(0, "/opt/trn_rl_repo")

import numpy as np

S, B, D = 1024, 64, 1024
NCORES = 8
BL = B // NCORES  # 8 batches per core
NCH = D // 128  # 8 partition chunks of the align dim

_compiled_nc = None


def _build_nc():
    from contextlib import ExitStack

    import concourse.bacc as bacc
    import concourse.tile as tile
    from concourse import mybir
    import concourse.bass as bass

    f32 = mybir.dt.float32
    f32r = mybir.dt.float32r
    AF = mybir.ActivationFunctionType

    nc = bacc.Bacc(
        "TRN2",
        target_bir_lowering=False,
        debug=False,
        enable_asserts=False,
        num_devices=NCORES,
    )

    # Inputs (per core, already sharded/transposed on host).
    uht = nc.dram_tensor("uht", [BL, D, S], f32, kind="ExternalInput").ap()  # (b,a,s)
    xsh = nc.dram_tensor("xsh", [BL, S, D], f32r, kind="ExternalInput").ap()  # (b,s,d)
    satw = nc.dram_tensor("satw", [D, D], f32r, kind="ExternalInput").ap()  # sa_w.T
    stm1t = nc.dram_tensor("stm1t", [D, BL], f32r, kind="ExternalInput").ap()  # (d,b)
    sabt = nc.dram_tensor("sabt", [128, NCH], f32, kind="ExternalInput").ap()
    # Masked matvec weights: a1wt4[p, c, j, m] = a1_w[c*128+p] if m == j else 0.
    a1wt4 = nc.dram_tensor("a1wt4", [128, NCH, 4, 4], f32r, kind="ExternalInput").ap()
    maskt = nc.dram_tensor("maskt", [4, 2, S], f32, kind="ExternalInput").ap()
    ident = nc.dram_tensor("ident", [128, 128], f32, kind="ExternalInput").ap()
    # Outputs: eijt[r, i, s] = e_ij_local[b = i*4 + r, s]; att likewise.
    eijt = nc.dram_tensor("eijt", [4, 2, S], f32, kind="ExternalOutput").ap()
    att = nc.dram_tensor("att", [4, 2, D], f32, kind="ExternalOutput").ap()

    with tile.TileContext(nc) as tc, ExitStack() as ctx:
        singles = ctx.enter_context(tc.tile_pool(name="singles", bufs=1))
        ubig = ctx.enter_context(tc.tile_pool(name="ubig", bufs=2))
        # satw shares the X slots: it is released after phase A, freeing a
        # third prefetch buffer for the xs_h stream.
        xbig = ctx.enter_context(tc.tile_pool(name="xbig", bufs=3))
        small = ctx.enter_context(tc.tile_pool(name="small", bufs=1))
        psum = ctx.enter_context(tc.tile_pool(name="psum", bufs=2, space="PSUM"))

        # ---- constants ----------------------------------------------------
        satw_sb = xbig.tile([128, NCH, D], f32r, tag="X", name="satw_sb")
        nc.sync.dma_start(out=satw_sb, in_=satw.rearrange("(k p) a -> p k a", p=128))
        stm1t_sb = singles.tile([128, NCH, BL], f32r, tag="stm1t")  # [d_p, d_k, b]
        nc.sync.dma_start(out=stm1t_sb, in_=stm1t.rearrange("(k p) b -> p k b", p=128))
        sabt_sb = singles.tile([128, NCH], f32, tag="sabt")
        nc.sync.dma_start(out=sabt_sb, in_=sabt)
        a1wt4_sb = singles.tile([128, NCH, 4, 4], f32r, tag="a1wt4")
        nc.sync.dma_start(out=a1wt4_sb, in_=a1wt4)
        ident_sb = singles.tile([128, 128], f32, tag="ident")
        nc.sync.dma_start(out=ident_sb, in_=ident)
        maskt_sb = singles.tile([4, 2, S], f32, tag="maskt")
        nc.sync.dma_start(out=maskt_sb, in_=maskt)

        # ---- sa_s^T = (s_tm1 @ sa_w.T).T + sa_b, laid out [a_p, ac*BL+b] --
        psum_sas = psum.tile([128, NCH * BL], f32, tag="ps", name="psum_sas")
        for ac in range(NCH):
            for k in range(NCH):
                nc.tensor.matmul(
                    psum_sas[:, ac * BL : (ac + 1) * BL],
                    lhsT=satw_sb[:, k, ac * 128 : (ac + 1) * 128],
                    rhs=stm1t_sb[:, k, :],
                    start=(k == 0),
                    stop=(k == NCH - 1),
                )
        sas_sb = singles.tile([128, NCH * BL], f32, tag="sas")
        for ac in range(NCH):
            nc.scalar.activation(
                sas_sb[:, ac * BL : (ac + 1) * BL],
                psum_sas[:, ac * BL : (ac + 1) * BL],
                AF.Identity,
                bias=sabt_sb[:, ac : ac + 1],
                scale=1.0,
            )

        # ---- e rows: tanh + f32r PE matvec over A -------------------------
        # Batch b accumulates into row b%4 of psum_e[b//4] via masked M=4
        # weights; rows of the other batches get zeros added.
        psum_e = [
            psum.tile([4, S], f32, tag="pe", name=f"psum_e{i}") for i in range(2)
        ]
        for b in range(BL):
            ut = ubig.tile([128, NCH, S], f32, tag="U", name="ut")  # [a_p, ac, s]
            nc.sync.dma_start(out=ut, in_=uht[b].rearrange("(c p) s -> p c s", p=128))
            pt = psum_e[b // 4]
            r = b % 4
            for c in range(NCH):
                ht = small.tile([128, S], f32r, tag="H", bufs=3, name="ht")
                nc.scalar.activation(
                    ht,
                    ut[:, c, :],
                    AF.Tanh,
                    bias=sas_sb[:, c * BL + b : c * BL + b + 1],
                    scale=1.0,
                )
                for h in range(2):
                    nc.tensor.matmul(
                        pt[0:4, h * 512 : (h + 1) * 512],
                        lhsT=a1wt4_sb[:, c, r, :],
                        rhs=ht[:, h * 512 : (h + 1) * 512],
                        start=(r == 0 and c == 0),
                        stop=(r == 3 and c == NCH - 1),
                    )

        # ---- masked softmax over S, per 4-batch group ---------------------
        eij = []
        for i in range(2):
            ea = small.tile([4, S], f32, tag="ea", bufs=2, name="ea")
            nc.scalar.copy(ea, psum_e[i][0:4, :])
            mx = small.tile([4, 1], f32, tag="mx", bufs=2, name="mx")
            nc.vector.reduce_max(out=mx, in_=ea, axis=mybir.AxisListType.X)
            nmx = small.tile([4, 1], f32, tag="nmx", bufs=2, name="nmx")
            nc.vector.tensor_scalar_mul(nmx, mx, -1.0)
            nc.scalar.activation(ea, ea, AF.Exp, bias=nmx, scale=1.0)
            nc.vector.tensor_mul(ea, ea, maskt_sb[:, i, :])
            sm = small.tile([4, 1], f32, tag="sm", bufs=2, name="sm")
            nc.vector.reduce_sum(out=sm, in_=ea, axis=mybir.AxisListType.X)
            rs = small.tile([4, 1], f32, tag="rs", bufs=2, name="rs")
            nc.vector.reciprocal(rs, sm)
            ej = small.tile([4, S], f32, tag="eij", bufs=2, name="ej")
            nc.vector.tensor_scalar_mul(ej, ea, rs)
            nc.gpsimd.dma_start(out=eijt[:, i, :], in_=ej)
            eij.append(ej)

        # ---- transpose e_ij into masked columns [s_p, (c, j, 4)] ----------
        # For group i, chunk c: out[p, m] for m=0..3 goes to the diagonal
        # positions m*4+m of the [4,4] block, giving lhsT[:, c, j, :] =
        # (e column of batch i*4+j at column j, zeros elsewhere).
        ecols4 = []
        for i in range(2):
            ptile = psum.tile([128, NCH, 16], f32, tag="pt", name=f"psum_t{i}")
            nc.vector.memset(ptile, 0.0)
            for c in range(NCH):
                blk = ptile[:, c, :]
                diag = bass.AP(
                    tensor=blk.tensor,
                    offset=blk.offset,
                    ap=[list(blk.ap[0]), [5, 4]],
                )
                nc.tensor.transpose(
                    diag, eij[i][:, c * 128 : (c + 1) * 128], ident_sb[0:4, 0:4]
                )
            ec = small.tile([128, NCH, 4, 4], f32r, tag="ecols", bufs=2, name="ec")
            nc.vector.tensor_copy(ec, ptile.rearrange("p c (j m) -> p c j m", m=4))
            ecols4.append(ec)

        # ---- attend: f32r PE matvec over S --------------------------------
        psum_a = [
            psum.tile([4, D], f32, tag="pe", name=f"psum_a{i}") for i in range(2)
        ]
        for b in range(BL):
            xt = xbig.tile([128, NCH, D], f32r, tag="X", name="xt")  # [s_p, sc, d]
            nc.sync.dma_start(out=xt, in_=xsh[b].rearrange("(c p) d -> p c d", p=128))
            pt = psum_a[b // 4]
            r = b % 4
            for c in range(NCH):
                for h in range(2):
                    nc.tensor.matmul(
                        pt[0:4, h * 512 : (h + 1) * 512],
                        lhsT=ecols4[b // 4][:, c, r, :],
                        rhs=xt[:, c, h * 512 : (h + 1) * 512],
                        start=(r == 0 and c == 0),
                        stop=(r == 3 and c == NCH - 1),
                    )
        for i in range(2):
            ats = small.tile([4, D], f32, tag="ats", bufs=2, name="ats")
            nc.scalar.copy(ats, psum_a[i][0:4, :])
            nc.gpsimd.dma_start(out=att[:, i, :], in_=ats)

    nc.compile()
    return nc


def get_nc():
    global _compiled_nc
    if _compiled_nc is None:
        _compiled_nc = _build_nc()
    return _compiled_nc


def make_in_maps(s_tm1, xs_h, uh, xs_mask, sa_w, sa_b, a1_w, a1_b):
    s_tm1 = np.asarray(s_tm1, dtype=np.float32)
    xs_h = np.asarray(xs_h, dtype=np.float32)
    uh = np.asarray(uh, dtype=np.float32)
    xs_mask = np.asarray(xs_mask, dtype=np.float32)
    sa_w = np.asarray(sa_w, dtype=np.float32)
    sa_b = np.asarray(sa_b, dtype=np.float32)
    a1_w = np.asarray(a1_w, dtype=np.float32)

    satw = np.ascontiguousarray(sa_w.T)
    sabt = np.ascontiguousarray(sa_b.reshape(NCH, 128).T)
    a1wt4 = np.zeros((128, NCH, 4, 4), dtype=np.float32)
    wcols = a1_w[0].reshape(NCH, 128).T  # [128, NCH]
    for j in range(4):
        a1wt4[:, :, j, j] = wcols
    ident = np.eye(128, dtype=np.float32)

    in_maps = []
    for i in range(NCORES):
        bs = slice(i * BL, (i + 1) * BL)
        maskt = np.ascontiguousarray(
            xs_mask[:, bs].T.reshape(2, 4, S).transpose(1, 0, 2)
        )
        in_maps.append(
            {
                "uht": np.ascontiguousarray(uh[:, bs, :].transpose(1, 2, 0)),
                "xsh": np.ascontiguousarray(xs_h[:, bs, :].transpose(1, 0, 2)),
                "satw": satw,
                "stm1t": np.ascontiguousarray(s_tm1[bs].T),
                "sabt": sabt,
                "a1wt4": a1wt4,
                "maskt": maskt,
                "ident": ident,
            }
        )
    return in_maps


def assemble_outputs(results):
    e_ij = np.empty((S, B), dtype=np.float32)
    attend = np.empty((B, D), dtype=np.float32)
    for i, r in enumerate(results):
        # eijt[r, g, s] = e_ij_local[b = g*4 + r, s]
        eloc = r["eijt"].transpose(1, 0, 2).reshape(BL, S)
        aloc = r["att"].transpose(1, 0, 2).reshape(BL, D)
        e_ij[:, i * BL : (i + 1) * BL] = eloc.T
        attend[i * BL : (i + 1) * BL, :] = aloc
    return e_ij, attend


def kernel(**inputs):
    from concourse.bass_utils import run_bass_kernel_spmd

    nc = get_nc()
    in_maps = make_in_maps(**inputs)
    res = run_bass_kernel_spmd(nc, in_maps, list(range(NCORES)))
    return assemble_outputs(res.results)


# revision 23
# speedup vs baseline: 1.4043x; 1.0647x over previous
"""Additive attention kernel for 8 TRN2 NeuronCores (Bass/Tile).

Problem (S=1024, B=64, D=1024 for all of dec/align/enc):
    sa_s   = s_tm1 @ sa_w.T + sa_b                  # [B, A]
    h      = tanh(sa_s[None] + uh)                  # [S, B, A]
    e      = einsum('sba,a->sb', h, a1_w[0]) + a1_b # [S, B]
    e_ij   = masked softmax of e over S             # [S, B]
    attend = einsum('sb,sbd->bd', e_ij, xs_h)       # [B, D]
    returns (e_ij, attend)

Sharding: data-parallel over B — each of the 8 cores handles 8 batches.
Device layout: the align dim A lives on SBUF partitions so the bias add fuses
into the ACT tanh as a per-partition bias, and both reductions (dot with a1_w
over A; weighted sum over S) are PE matvecs in float32r (1 cycle/row vs 4 for
fp32).

float32r matmuls reject column tile_position offsets, so each batch's matvec
uses M=4 stationary weights with the weight vector in column b%4 and zeros
elsewhere: row b%4 of the [4, N] PSUM output gets the real dot product, the
other rows accumulate zeros. The 4 rows of each PSUM tile land at partitions
0..3, which every later engine op can address (SBUF engine operands must start
at partition 0/32/64/96).

Host-side sharding pre-transposes uh to (b, a, s) and xs_h to (b, s, d) so all
device DMAs are dense 4 KiB-per-partition line loads; small outputs are
returned in device-friendly layouts and rearranged on host.
"""

import sys

if "/opt/trn_rl_repo" not in sys.path:
    sys.path.insert(0, "/opt/trn_rl_repo")

import numpy as np

S, B, D = 1024, 64, 1024
NCORES = 8
BL = B // NCORES  # 8 batches per core
NCH = D // 128  # 8 partition chunks of the align dim

_compiled_nc = None


def _build_nc():
    from contextlib import ExitStack

    import concourse.bacc as bacc
    import concourse.tile as tile
    from concourse import mybir
    import concourse.bass as bass

    f32 = mybir.dt.float32
    f32r = mybir.dt.float32r
    AF = mybir.ActivationFunctionType

    nc = bacc.Bacc(
        "TRN2",
        target_bir_lowering=False,
        debug=False,
        enable_asserts=False,
        num_devices=NCORES,
    )

    # Inputs (per core, already sharded/transposed on host).
    uht = nc.dram_tensor("uht", [BL, 128, NCH, S], f32, kind="ExternalInput").ap()
    xsh = nc.dram_tensor("xsh", [BL, 128, NCH, D], f32r, kind="ExternalInput").ap()
    satw = nc.dram_tensor("satw", [D, D], f32r, kind="ExternalInput").ap()  # sa_w.T
    stm1t = nc.dram_tensor("stm1t", [D, BL], f32r, kind="ExternalInput").ap()  # (d,b)
    sabt = nc.dram_tensor("sabt", [128, NCH], f32, kind="ExternalInput").ap()
    # Masked matvec weights: a1wt4[p, c, j, m] = a1_w[c*128+p] if m == j else 0.
    a1wt4 = nc.dram_tensor("a1wt4", [128, NCH, 4, 4], f32r, kind="ExternalInput").ap()
    maskt = nc.dram_tensor("maskt", [4, 2, S], f32, kind="ExternalInput").ap()
    ident = nc.dram_tensor("ident", [128, 128], f32, kind="ExternalInput").ap()
    # Outputs: eijt[r, i, s] = e_ij_local[b = i*4 + r, s]; att likewise.
    eijt = nc.dram_tensor("eijt", [4, 2, S], f32, kind="ExternalOutput").ap()
    att = nc.dram_tensor("att", [4, 2, D], f32, kind="ExternalOutput").ap()

    with tile.TileContext(nc) as tc, ExitStack() as ctx:
        singles = ctx.enter_context(tc.tile_pool(name="singles", bufs=1))
        ubig = ctx.enter_context(tc.tile_pool(name="ubig", bufs=2))
        # satw shares the X slots: it is released after phase A, freeing a
        # third prefetch buffer for the xs_h stream.
        xbig = ctx.enter_context(tc.tile_pool(name="xbig", bufs=3))
        small = ctx.enter_context(tc.tile_pool(name="small", bufs=1))
        psum = ctx.enter_context(tc.tile_pool(name="psum", bufs=2, space="PSUM"))

        # ---- constants ----------------------------------------------------
        satw_sb = xbig.tile([128, NCH, D], f32r, tag="X", name="satw_sb")
        nc.sync.dma_start(out=satw_sb, in_=satw.rearrange("(k p) a -> p k a", p=128))
        stm1t_sb = singles.tile([128, NCH, BL], f32r, tag="stm1t")  # [d_p, d_k, b]
        nc.sync.dma_start(out=stm1t_sb, in_=stm1t.rearrange("(k p) b -> p k b", p=128))
        sabt_sb = singles.tile([128, NCH], f32, tag="sabt")
        nc.sync.dma_start(out=sabt_sb, in_=sabt)
        a1wt4_sb = singles.tile([128, NCH, 4, 4], f32r, tag="a1wt4")
        nc.sync.dma_start(out=a1wt4_sb, in_=a1wt4)
        ident_sb = singles.tile([128, 128], f32, tag="ident")
        nc.sync.dma_start(out=ident_sb, in_=ident)
        maskt_sb = singles.tile([4, 2, S], f32, tag="maskt")
        nc.sync.dma_start(out=maskt_sb, in_=maskt)

        # ---- sa_s^T = (s_tm1 @ sa_w.T).T + sa_b, laid out [a_p, ac*BL+b] --
        psum_sas = psum.tile([128, NCH * BL], f32, tag="ps", name="psum_sas")
        for ac in range(NCH):
            for k in range(NCH):
                nc.tensor.matmul(
                    psum_sas[:, ac * BL : (ac + 1) * BL],
                    lhsT=satw_sb[:, k, ac * 128 : (ac + 1) * 128],
                    rhs=stm1t_sb[:, k, :],
                    start=(k == 0),
                    stop=(k == NCH - 1),
                )
        sas_sb = singles.tile([128, NCH * BL], f32, tag="sas")
        for ac in range(NCH):
            nc.scalar.activation(
                sas_sb[:, ac * BL : (ac + 1) * BL],
                psum_sas[:, ac * BL : (ac + 1) * BL],
                AF.Identity,
                bias=sabt_sb[:, ac : ac + 1],
                scale=1.0,
            )

        # ---- e rows: tanh + f32r PE matvec over A -------------------------
        # Batch b accumulates into row b%4 of psum_e[b//4] via masked M=4
        # weights; rows of the other batches get zeros added.
        psum_e = [
            psum.tile([4, S], f32, tag="pe", name=f"psum_e{i}") for i in range(2)
        ]
        for b in range(BL):
            ut = ubig.tile([128, NCH, S], f32, tag="U", name="ut")  # [a_p, ac, s]
            nc.sync.dma_start(out=ut, in_=uht[b])
            pt = psum_e[b // 4]
            r = b % 4
            for c in range(NCH):
                ht = small.tile([128, S], f32r, tag="H", bufs=3, name="ht")
                nc.scalar.activation(
                    ht,
                    ut[:, c, :],
                    AF.Tanh,
                    bias=sas_sb[:, c * BL + b : c * BL + b + 1],
                    scale=1.0,
                )
                for h in range(2):
                    nc.tensor.matmul(
                        pt[0:4, h * 512 : (h + 1) * 512],
                        lhsT=a1wt4_sb[:, c, r, :],
                        rhs=ht[:, h * 512 : (h + 1) * 512],
                        start=(r == 0 and c == 0),
                        stop=(r == 3 and c == NCH - 1),
                    )

        # ---- masked softmax over S, per 4-batch group ---------------------
        eij = []
        for i in range(2):
            ea = small.tile([4, S], f32, tag="ea", bufs=2, name="ea")
            nc.scalar.copy(ea, psum_e[i][0:4, :])
            mx = small.tile([4, 1], f32, tag="mx", bufs=2, name="mx")
            nc.vector.reduce_max(out=mx, in_=ea, axis=mybir.AxisListType.X)
            nmx = small.tile([4, 1], f32, tag="nmx", bufs=2, name="nmx")
            nc.vector.tensor_scalar_mul(nmx, mx, -1.0)
            nc.scalar.activation(ea, ea, AF.Exp, bias=nmx, scale=1.0)
            nc.vector.tensor_mul(ea, ea, maskt_sb[:, i, :])
            sm = small.tile([4, 1], f32, tag="sm", bufs=2, name="sm")
            nc.vector.reduce_sum(out=sm, in_=ea, axis=mybir.AxisListType.X)
            rs = small.tile([4, 1], f32, tag="rs", bufs=2, name="rs")
            nc.vector.reciprocal(rs, sm)
            ej = small.tile([4, S], f32, tag="eij", bufs=2, name="ej")
            nc.vector.tensor_scalar_mul(ej, ea, rs)
            nc.gpsimd.dma_start(out=eijt[:, i, :], in_=ej)
            eij.append(ej)

        # ---- transpose e_ij into masked columns [s_p, (c, j, 4)] ----------
        # For group i, chunk c: out[p, m] for m=0..3 goes to the diagonal
        # positions m*4+m of the [4,4] block, giving lhsT[:, c, j, :] =
        # (e column of batch i*4+j at column j, zeros elsewhere).
        ecols4 = []
        for i in range(2):
            ptile = psum.tile([128, NCH, 16], f32, tag="pt", name=f"psum_t{i}")
            nc.vector.memset(ptile, 0.0)
            for c in range(NCH):
                blk = ptile[:, c, :]
                diag = bass.AP(
                    tensor=blk.tensor,
                    offset=blk.offset,
                    ap=[list(blk.ap[0]), [5, 4]],
                )
                nc.tensor.transpose(
                    diag, eij[i][:, c * 128 : (c + 1) * 128], ident_sb[0:4, 0:4]
                )
            ec = small.tile([128, NCH, 4, 4], f32r, tag="ecols", bufs=2, name="ec")
            nc.vector.tensor_copy(ec, ptile.rearrange("p c (j m) -> p c j m", m=4))
            ecols4.append(ec)

        # ---- attend: f32r PE matvec over S --------------------------------
        psum_a = [
            psum.tile([4, D], f32, tag="pe", name=f"psum_a{i}") for i in range(2)
        ]
        for b in range(BL):
            xt = xbig.tile([128, NCH, D], f32r, tag="X", name="xt")  # [s_p, sc, d]
            nc.sync.dma_start(out=xt, in_=xsh[b])
            pt = psum_a[b // 4]
            r = b % 4
            for c in range(NCH):
                for h in range(2):
                    nc.tensor.matmul(
                        pt[0:4, h * 512 : (h + 1) * 512],
                        lhsT=ecols4[b // 4][:, c, r, :],
                        rhs=xt[:, c, h * 512 : (h + 1) * 512],
                        start=(r == 0 and c == 0),
                        stop=(r == 3 and c == NCH - 1),
                    )
            if r == 3:
                i = b // 4
                ats = small.tile([4, D], f32, tag="ats", bufs=2, name="ats")
                nc.scalar.copy(ats, psum_a[i][0:4, :])
                nc.gpsimd.dma_start(out=att[:, i, :], in_=ats)

    nc.compile()
    return nc


def get_nc():
    global _compiled_nc
    if _compiled_nc is None:
        _compiled_nc = _build_nc()
    return _compiled_nc


def make_in_maps(s_tm1, xs_h, uh, xs_mask, sa_w, sa_b, a1_w, a1_b):
    s_tm1 = np.asarray(s_tm1, dtype=np.float32)
    xs_h = np.asarray(xs_h, dtype=np.float32)
    uh = np.asarray(uh, dtype=np.float32)
    xs_mask = np.asarray(xs_mask, dtype=np.float32)
    sa_w = np.asarray(sa_w, dtype=np.float32)
    sa_b = np.asarray(sa_b, dtype=np.float32)
    a1_w = np.asarray(a1_w, dtype=np.float32)

    satw = np.ascontiguousarray(sa_w.T)
    sabt = np.ascontiguousarray(sa_b.reshape(NCH, 128).T)
    a1wt4 = np.zeros((128, NCH, 4, 4), dtype=np.float32)
    wcols = a1_w[0].reshape(NCH, 128).T  # [128, NCH]
    for j in range(4):
        a1wt4[:, :, j, j] = wcols
    ident = np.eye(128, dtype=np.float32)

    in_maps = []
    for i in range(NCORES):
        bs = slice(i * BL, (i + 1) * BL)
        maskt = np.ascontiguousarray(
            xs_mask[:, bs].T.reshape(2, 4, S).transpose(1, 0, 2)
        )
        in_maps.append(
            {
                "uht": np.ascontiguousarray(
                    uh[:, bs, :].transpose(1, 2, 0).reshape(BL, NCH, 128, S).transpose(0, 2, 1, 3)
                ),
                "xsh": np.ascontiguousarray(
                    xs_h[:, bs, :].transpose(1, 0, 2).reshape(BL, NCH, 128, D).transpose(0, 2, 1, 3)
                ),
                "satw": satw,
                "stm1t": np.ascontiguousarray(s_tm1[bs].T),
                "sabt": sabt,
                "a1wt4": a1wt4,
                "maskt": maskt,
                "ident": ident,
            }
        )
    return in_maps


def assemble_outputs(results):
    e_ij = np.empty((S, B), dtype=np.float32)
    attend = np.empty((B, D), dtype=np.float32)
    for i, r in enumerate(results):
        # eijt[r, g, s] = e_ij_local[b = g*4 + r, s]
        eloc = r["eijt"].transpose(1, 0, 2).reshape(BL, S)
        aloc = r["att"].transpose(1, 0, 2).reshape(BL, D)
        e_ij[:, i * BL : (i + 1) * BL] = eloc.T
        attend[i * BL : (i + 1) * BL, :] = aloc
    return e_ij, attend


def kernel(**inputs):
    from concourse.bass_utils import run_bass_kernel_spmd

    nc = get_nc()
    in_maps = make_in_maps(**inputs)
    res = run_bass_kernel_spmd(nc, in_maps, list(range(NCORES)))
    return assemble_outputs(res.results)


# revision 24
# speedup vs baseline: 1.6695x; 1.1888x over previous
"""Additive attention kernel for 8 TRN2 NeuronCores (Bass/Tile).

Problem (S=1024, B=64, D=1024 for all of dec/align/enc):
    sa_s   = s_tm1 @ sa_w.T + sa_b                  # [B, A]
    h      = tanh(sa_s[None] + uh)                  # [S, B, A]
    e      = einsum('sba,a->sb', h, a1_w[0]) + a1_b # [S, B]
    e_ij   = masked softmax of e over S             # [S, B]
    attend = einsum('sb,sbd->bd', e_ij, xs_h)       # [B, D]
    returns (e_ij, attend)

Sharding: data-parallel over B — each of the 8 cores handles 8 batches.
Device layout: the align dim A lives on SBUF partitions so the bias add fuses
into the ACT tanh as a per-partition bias, and both reductions (dot with a1_w
over A; weighted sum over S) are PE matvecs in float32r (1 cycle/row vs 4 for
fp32).

float32r matmuls reject column tile_position offsets, so each batch's matvec
uses M=4 stationary weights with the weight vector in column b%4 and zeros
elsewhere: row b%4 of the [4, N] PSUM output gets the real dot product, the
other rows accumulate zeros. The 4 rows of each PSUM tile land at partitions
0..3, which every later engine op can address (SBUF engine operands must start
at partition 0/32/64/96).

Host-side sharding pre-transposes uh to (b, a, s) and xs_h to (b, s, d) so all
device DMAs are dense 4 KiB-per-partition line loads; small outputs are
returned in device-friendly layouts and rearranged on host.
"""

import sys

if "/opt/trn_rl_repo" not in sys.path:
    sys.path.insert(0, "/opt/trn_rl_repo")

import numpy as np

S, B, D = 1024, 64, 1024
NCORES = 8
BL = B // NCORES  # 8 batches per core
NCH = D // 128  # 8 partition chunks of the align dim

_compiled_nc = None


def _build_nc():
    from contextlib import ExitStack

    import concourse.bacc as bacc
    import concourse.tile as tile
    from concourse import mybir
    import concourse.bass as bass

    f32 = mybir.dt.float32
    f32r = mybir.dt.float32r
    AF = mybir.ActivationFunctionType

    nc = bacc.Bacc(
        "TRN2",
        target_bir_lowering=False,
        debug=False,
        enable_asserts=False,
        num_devices=NCORES,
    )

    # Inputs (per core, already sharded/transposed on host).
    uht = nc.dram_tensor("uht", [BL, 128, NCH, S], f32, kind="ExternalInput").ap()
    xsh = nc.dram_tensor("xsh", [BL, 128, NCH, D], f32r, kind="ExternalInput").ap()
    satw = nc.dram_tensor("satw", [D, D], f32r, kind="ExternalInput").ap()  # sa_w.T
    stm1t = nc.dram_tensor("stm1t", [D, BL], f32r, kind="ExternalInput").ap()  # (d,b)
    sabt = nc.dram_tensor("sabt", [128, NCH], f32, kind="ExternalInput").ap()
    # Masked matvec weights: a1wt4[p, c, j, m] = a1_w[c*128+p] if m == j else 0.
    a1wt4 = nc.dram_tensor("a1wt4", [128, NCH, 4, 4], f32r, kind="ExternalInput").ap()
    maskt = nc.dram_tensor("maskt", [4, 2, S], f32, kind="ExternalInput").ap()
    ident = nc.dram_tensor("ident", [128, 128], f32, kind="ExternalInput").ap()
    # Outputs: eijt[r, i, s] = e_ij_local[b = i*4 + r, s]; att likewise.
    eijt = nc.dram_tensor("eijt", [4, 2, S], f32, kind="ExternalOutput").ap()
    att = nc.dram_tensor("att", [4, 2, D], f32, kind="ExternalOutput").ap()

    with tile.TileContext(nc) as tc, ExitStack() as ctx:
        singles = ctx.enter_context(tc.tile_pool(name="singles", bufs=1))
        ubig = ctx.enter_context(tc.tile_pool(name="ubig", bufs=2))
        # satw shares the X slots: it is released after phase A, freeing a
        # third prefetch buffer for the xs_h stream.
        xbig = ctx.enter_context(tc.tile_pool(name="xbig", bufs=3))
        small = ctx.enter_context(tc.tile_pool(name="small", bufs=1))
        psum = ctx.enter_context(tc.tile_pool(name="psum", bufs=2, space="PSUM"))

        # ---- constants ----------------------------------------------------
        satw_sb = xbig.tile([128, NCH, D], f32r, tag="X", name="satw_sb")
        nc.sync.dma_start(out=satw_sb, in_=satw.rearrange("(k p) a -> p k a", p=128))
        stm1t_sb = singles.tile([128, NCH, BL], f32r, tag="stm1t")  # [d_p, d_k, b]
        nc.sync.dma_start(out=stm1t_sb, in_=stm1t.rearrange("(k p) b -> p k b", p=128))
        sabt_sb = singles.tile([128, NCH], f32, tag="sabt")
        nc.sync.dma_start(out=sabt_sb, in_=sabt)
        a1wt4_sb = singles.tile([128, NCH, 4, 4], f32r, tag="a1wt4")
        nc.sync.dma_start(out=a1wt4_sb, in_=a1wt4)
        ident_sb = singles.tile([128, 128], f32, tag="ident")
        nc.sync.dma_start(out=ident_sb, in_=ident)
        maskt_sb = singles.tile([4, 2, S], f32, tag="maskt")
        nc.sync.dma_start(out=maskt_sb, in_=maskt)

        # ---- sa_s^T = (s_tm1 @ sa_w.T).T + sa_b, laid out [a_p, ac*BL+b] --
        psum_sas = psum.tile([128, NCH * BL], f32, tag="ps", name="psum_sas")
        for ac in range(NCH):
            for k in range(NCH):
                nc.tensor.matmul(
                    psum_sas[:, ac * BL : (ac + 1) * BL],
                    lhsT=satw_sb[:, k, ac * 128 : (ac + 1) * 128],
                    rhs=stm1t_sb[:, k, :],
                    start=(k == 0),
                    stop=(k == NCH - 1),
                )
        sas_sb = singles.tile([128, NCH * BL], f32, tag="sas")
        for ac in range(NCH):
            nc.scalar.activation(
                sas_sb[:, ac * BL : (ac + 1) * BL],
                psum_sas[:, ac * BL : (ac + 1) * BL],
                AF.Identity,
                bias=sabt_sb[:, ac : ac + 1],
                scale=1.0,
            )

        # ---- e rows: tanh + f32r PE matvec over A -------------------------
        # Processed in two groups of 4 batches; each group runs
        # E-phase -> softmax -> transpose -> attend, so group 0's attend
        # compute overlaps group 1's uh stream.
        for i in range(2):
            # Batch b accumulates into row b%4 of psum_eg via masked M=4
            # weights; rows of the other batches get zeros added.
            psum_eg = psum.tile([4, S], f32, tag="pe", name=f"psum_e{i}")
            for r in range(4):
                b = 4 * i + r
                ut = ubig.tile([128, NCH, S], f32, tag="U", name="ut")
                nc.sync.dma_start(out=ut[:, 0:4, :], in_=uht[b, :, 0:4, :])
                nc.sync.dma_start(out=ut[:, 4:8, :], in_=uht[b, :, 4:8, :])
                for c in range(NCH):
                    ht = small.tile([128, S], f32r, tag="H", bufs=3, name="ht")
                    nc.scalar.activation(
                        ht,
                        ut[:, c, :],
                        AF.Tanh,
                        bias=sas_sb[:, c * BL + b : c * BL + b + 1],
                        scale=1.0,
                    )
                    for h in range(2):
                        nc.tensor.matmul(
                            psum_eg[0:4, h * 512 : (h + 1) * 512],
                            lhsT=a1wt4_sb[:, c, r, :],
                            rhs=ht[:, h * 512 : (h + 1) * 512],
                            start=(r == 0 and c == 0),
                            stop=(r == 3 and c == NCH - 1),
                        )

            # ---- masked softmax over S for this group ---------------------
            ea = small.tile([4, S], f32, tag="ea", bufs=2, name="ea")
            nc.scalar.copy(ea, psum_eg[0:4, :])
            mx = small.tile([4, 1], f32, tag="mx", bufs=2, name="mx")
            nc.vector.reduce_max(out=mx, in_=ea, axis=mybir.AxisListType.X)
            nmx = small.tile([4, 1], f32, tag="nmx", bufs=2, name="nmx")
            nc.vector.tensor_scalar_mul(nmx, mx, -1.0)
            nc.scalar.activation(ea, ea, AF.Exp, bias=nmx, scale=1.0)
            nc.vector.tensor_mul(ea, ea, maskt_sb[:, i, :])
            sm = small.tile([4, 1], f32, tag="sm", bufs=2, name="sm")
            nc.vector.reduce_sum(out=sm, in_=ea, axis=mybir.AxisListType.X)
            rs = small.tile([4, 1], f32, tag="rs", bufs=2, name="rs")
            nc.vector.reciprocal(rs, sm)
            ej = small.tile([4, S], f32, tag="eij", bufs=2, name="ej")
            nc.vector.tensor_scalar_mul(ej, ea, rs)
            nc.gpsimd.dma_start(out=eijt[:, i, :], in_=ej)

            # ---- transpose e_ij into masked columns [s_p, (c, j, 4)] ------
            # For chunk c: out[p, m] goes to diagonal position m*4+m of the
            # [4,4] block, giving lhsT[:, c, j, :] = (e column of batch
            # 4*i+j at column j, zeros elsewhere).
            ptile = psum.tile([128, NCH, 16], f32, tag="pt", name=f"psum_t{i}")
            nc.vector.memset(ptile, 0.0)
            for c in range(NCH):
                blk = ptile[:, c, :]
                diag = bass.AP(
                    tensor=blk.tensor,
                    offset=blk.offset,
                    ap=[list(blk.ap[0]), [5, 4]],
                )
                nc.tensor.transpose(
                    diag, ej[:, c * 128 : (c + 1) * 128], ident_sb[0:4, 0:4]
                )
            ec = small.tile([128, NCH, 4, 4], f32r, tag="ecols", bufs=2, name="ec")
            nc.vector.tensor_copy(ec, ptile.rearrange("p c (j m) -> p c j m", m=4))

            # ---- attend: f32r PE matvec over S ----------------------------
            psum_ag = psum.tile([4, D], f32, tag="pe", name=f"psum_a{i}")
            for r in range(4):
                b = 4 * i + r
                xt = xbig.tile([128, NCH, D], f32r, tag="X", name="xt")
                nc.sync.dma_start(out=xt[:, 0:4, :], in_=xsh[b, :, 0:4, :])
                nc.sync.dma_start(out=xt[:, 4:8, :], in_=xsh[b, :, 4:8, :])
                for c in range(NCH):
                    for h in range(2):
                        nc.tensor.matmul(
                            psum_ag[0:4, h * 512 : (h + 1) * 512],
                            lhsT=ec[:, c, r, :],
                            rhs=xt[:, c, h * 512 : (h + 1) * 512],
                            start=(r == 0 and c == 0),
                            stop=(r == 3 and c == NCH - 1),
                        )
            ats = small.tile([4, D], f32, tag="ats", bufs=2, name="ats")
            nc.scalar.copy(ats, psum_ag[0:4, :])
            nc.gpsimd.dma_start(out=att[:, i, :], in_=ats)

    nc.compile()
    return nc


def get_nc():
    global _compiled_nc
    if _compiled_nc is None:
        _compiled_nc = _build_nc()
    return _compiled_nc


def make_in_maps(s_tm1, xs_h, uh, xs_mask, sa_w, sa_b, a1_w, a1_b):
    s_tm1 = np.asarray(s_tm1, dtype=np.float32)
    xs_h = np.asarray(xs_h, dtype=np.float32)
    uh = np.asarray(uh, dtype=np.float32)
    xs_mask = np.asarray(xs_mask, dtype=np.float32)
    sa_w = np.asarray(sa_w, dtype=np.float32)
    sa_b = np.asarray(sa_b, dtype=np.float32)
    a1_w = np.asarray(a1_w, dtype=np.float32)

    satw = np.ascontiguousarray(sa_w.T)
    sabt = np.ascontiguousarray(sa_b.reshape(NCH, 128).T)
    a1wt4 = np.zeros((128, NCH, 4, 4), dtype=np.float32)
    wcols = a1_w[0].reshape(NCH, 128).T  # [128, NCH]
    for j in range(4):
        a1wt4[:, :, j, j] = wcols
    ident = np.eye(128, dtype=np.float32)

    in_maps = []
    for i in range(NCORES):
        bs = slice(i * BL, (i + 1) * BL)
        maskt = np.ascontiguousarray(
            xs_mask[:, bs].T.reshape(2, 4, S).transpose(1, 0, 2)
        )
        in_maps.append(
            {
                "uht": np.ascontiguousarray(
                    uh[:, bs, :].transpose(1, 2, 0).reshape(BL, NCH, 128, S).transpose(0, 2, 1, 3)
                ),
                "xsh": np.ascontiguousarray(
                    xs_h[:, bs, :].transpose(1, 0, 2).reshape(BL, NCH, 128, D).transpose(0, 2, 1, 3)
                ),
                "satw": satw,
                "stm1t": np.ascontiguousarray(s_tm1[bs].T),
                "sabt": sabt,
                "a1wt4": a1wt4,
                "maskt": maskt,
                "ident": ident,
            }
        )
    return in_maps


def assemble_outputs(results):
    e_ij = np.empty((S, B), dtype=np.float32)
    attend = np.empty((B, D), dtype=np.float32)
    for i, r in enumerate(results):
        # eijt[r, g, s] = e_ij_local[b = g*4 + r, s]
        eloc = r["eijt"].transpose(1, 0, 2).reshape(BL, S)
        aloc = r["att"].transpose(1, 0, 2).reshape(BL, D)
        e_ij[:, i * BL : (i + 1) * BL] = eloc.T
        attend[i * BL : (i + 1) * BL, :] = aloc
    return e_ij, attend


def kernel(**inputs):
    from concourse.bass_utils import run_bass_kernel_spmd

    nc = get_nc()
    in_maps = make_in_maps(**inputs)
    res = run_bass_kernel_spmd(nc, in_maps, list(range(NCORES)))
    return assemble_outputs(res.results)
